# revision 20
# baseline (speedup 1.0000x reference)
"""Trainium2 Bass kernel for nn_ConditionEncoder (sparse-conv UNet encoder).

Strategy: shard output voxels (rows) of every sparse conv across 8 NeuronCores
in contiguous blocks; replicate full feature tables in per-core DRAM via
AllGather between layers; gather-GEMM each conv as
  gather 256B rows (indirect DMA) -> PE transpose to channel-major ->
  K=128 two-tap-packed f32r matmuls accumulating in PSUM -> bias(+relu) ->
  PE transpose back to row-major -> DMA to the next table.
The pre conv (Cin=4) is fed by a host-side pre-gathered, channel-major,
27-tap-stacked operand so it is a pure dense matmul on device.
"""
import sys
import numpy as np

sys.path.insert(0, '/opt/trn_rl_repo')

NCORES = 8
F = 64
S = 8
NLVL = [256000, 64000, 16000, 4000]
NLOC = [n // NCORES for n in NLVL]            # 32000, 8000, 2000, 500
NPAD = [32256, 8192, 2048, 512]               # per-core padded block (mult of 512)
# NB: 8*32256*64 + 64 = 16,515,136 elements < 2^24 — the walrus indirect-DMA
# codegen appears to assert on larger source tensors.
NT = 512                                      # outputs per tile
NPAIR = 14                                    # 28 padded taps / 2

_CACHE = {}


def _conv_specs():
    # (name, lvl_in, lvl_out, table_in, table_out, Cout, relu, is_bg, q_out)
    return [
        ("down0", 0, 1, "T0", "TA1", F, False, False, False),
        ("c1_0", 1, 1, "TA1", "TB1", F, True, False, False),
        ("c2_0", 1, 1, "TB1", "TC1", F, False, False, False),
        ("p1_0", 1, 1, "TC1", "TP1", F, True, False, False),
        ("p2_0", 1, 1, "TP1", None, S, False, True, False),
        ("down1", 1, 2, "TC1", "TA2", F, False, False, False),
        ("c1_1", 2, 2, "TA2", "TB2", F, True, False, False),
        ("c2_1", 2, 2, "TB2", "TC2", F, False, False, False),
        ("p1_1", 2, 2, "TC2", "TP2", F, True, False, False),
        ("p2_1", 2, 2, "TP2", None, S, False, True, False),
        ("down2", 2, 3, "TC2", "TA3", F, False, False, False),
        ("c1_2", 3, 3, "TA3", "TB3", F, True, False, False),
        ("c2_2", 3, 3, "TB3", "TC3", F, False, False, True),
        ("p1_2", 3, 3, "TC3", "TP3", F, True, False, False),
        ("p2_2", 3, 3, "TP3", None, S, False, True, False),
    ]


def _build():
    import concourse.bacc as bacc
    import concourse.tile as tile
    from concourse import bass, mybir
    from concourse.masks import make_identity

    nc = bacc.Bacc("TRN2", target_bir_lowering=False, debug=False,
                   num_devices=NCORES)
    RG = [list(range(NCORES))]

    # ---- External inputs ----
    x27 = nc.dram_tensor("x27", [128, NPAD[0]], mybir.dt.float32, kind="ExternalInput")
    prew = nc.dram_tensor("prew", [128, F], mybir.dt.float32, kind="ExternalInput")
    preb = nc.dram_tensor("preb", [F, 1], mybir.dt.float32, kind="ExternalInput")
    specs = _conv_specs()
    idx_in, w_in, b_in = {}, {}, {}
    for (name, li, lo, tin, tout, cout, relu, isbg, isq) in specs:
        npo = NPAD[lo]
        idx_in[name] = nc.dram_tensor(f"idx_{name}", [128, NPAIR * (npo // NT) * 8],
                                      mybir.dt.int32, kind="ExternalInput")
        w_in[name] = nc.dram_tensor(f"w_{name}", [128, NPAIR * cout],
                                    mybir.dt.float32, kind="ExternalInput")
        b_in[name] = nc.dram_tensor(f"b_{name}", [cout, 1],
                                    mybir.dt.float32, kind="ExternalInput")

    # ---- External outputs (per-core blocks) ----
    q_out = nc.dram_tensor("q_out", [NPAD[3], F], mybir.dt.float32, kind="ExternalOutput")
    import os
    DEBUG = bool(os.environ.get("KERNEL_DEBUG"))
    dbg = {}
    if DEBUG:
        dbg["pre"] = nc.dram_tensor("dbg_pre", [NPAD[0], F], mybir.dt.float32, kind="ExternalOutput")
        dbg["down0"] = nc.dram_tensor("dbg_down0", [NPAD[1], F], mybir.dt.float32, kind="ExternalOutput")
        dbg["c1_0"] = nc.dram_tensor("dbg_c1_0", [NPAD[1], F], mybir.dt.float32, kind="ExternalOutput")
    bg_out = {
        "p2_0": nc.dram_tensor("bg0_out", [NPAD[1], S], mybir.dt.float32, kind="ExternalOutput"),
        "p2_1": nc.dram_tensor("bg1_out", [NPAD[2], S], mybir.dt.float32, kind="ExternalOutput"),
        "p2_2": nc.dram_tensor("bg2_out", [NPAD[3], S], mybir.dt.float32, kind="ExternalOutput"),
    }

    with tile.TileContext(nc) as tc:
        with tc.tile_pool(name="dram", bufs=1, space="DRAM") as dp, \
             tc.tile_pool(name="cst", bufs=1) as cst, \
             tc.tile_pool(name="sb", bufs=3) as sb, \
             tc.tile_pool(name="ps", bufs=2, space="PSUM") as ps:

            # persistent DRAM tables (full, gathered) and per-core out blocks
            tables = {}
            for lvl, names in [(0, ["T0"]), (1, ["TA1", "TB1", "TC1", "TP1"]),
                               (2, ["TA2", "TB2", "TC2", "TP2"]),
                               (3, ["TA3", "TB3", "TC3", "TP3"])]:
                for nm in names:
                    tables[nm] = dp.tile([NCORES * NPAD[lvl] + 1, F],
                                         mybir.dt.float32, tag=nm, name=f"tbl_{nm}")
            myout = {}
            for lvl in (1, 2, 3):
                myout[lvl] = dp.tile([NPAD[lvl], F], mybir.dt.float32,
                                     tag=f"my{lvl}", name=f"my{lvl}")
            myout0 = dp.tile([NPAD[0], F], mybir.dt.float32, tag="my0")

            # identities for PE transposes
            id128 = cst.tile([128, 128], mybir.dt.float32, tag="id128")
            make_identity(nc, id128[:])
            id64 = cst.tile([F, F], mybir.dt.float32, tag="id64")
            make_identity(nc, id64[:])
            id8 = cst.tile([S, S], mybir.dt.float32, tag="id8")
            make_identity(nc, id8[:])

            # zero row for every table
            zrow = cst.tile([1, F], mybir.dt.float32, tag="zrow")
            nc.vector.memset(zrow[:], 0.0)
            for nm, t in tables.items():
                nrows = t.shape[0]
                nc.sync.dma_start(t[nrows - 1:nrows, :], zrow[:])

            def epilogue(psO, cout, relu, n_chunks, dests):
                """psO [cout, NT] channel-major -> bias(+relu) -> row-major -> DMA."""
                sT = sb.tile([cout, NT], mybir.dt.float32, tag=f"sT{cout}")
                nc.scalar.activation(
                    sT[:], psO[:],
                    mybir.ActivationFunctionType.Relu if relu
                    else mybir.ActivationFunctionType.Identity,
                    bias=biast[:])
                psN = ps.tile([128, n_chunks * cout], mybir.dt.float32, tag="psN")
                ident = id64 if cout == F else id8
                for m in range(n_chunks):
                    nc.tensor.transpose(psN[:, m * cout:(m + 1) * cout],
                                        sT[:, m * 128:(m + 1) * 128], ident[:])
                sN = sb.tile([128, n_chunks * cout], mybir.dt.float32, tag="sN")
                nc.vector.tensor_copy(out=sN[:], in_=psN[:])
                for dst in dests:
                    nc.sync.dma_start(dst, sN[:].rearrange("p (m c) -> p m c", m=n_chunks))

            # ---------------- pre conv ----------------
            prew_t = cst.tile([128, F], mybir.dt.float32r, tag="prew")
            prew_s = cst.tile([128, F], mybir.dt.float32, tag="prew32")
            nc.sync.dma_start(prew_s[:], prew.ap())
            nc.vector.tensor_copy(out=prew_t[:], in_=prew_s[:])
            biast = cst.tile([F, 1], mybir.dt.float32, tag="bias")
            nc.sync.dma_start(biast[:], preb.ap())
            for t in range(NPAD[0] // NT):
                rhs32 = sb.tile([128, NT], mybir.dt.float32, tag="prerhs32")
                nc.sync.dma_start(rhs32[:], x27.ap()[:, t * NT:(t + 1) * NT])
                rhs = sb.tile([128, NT], mybir.dt.float32r, tag="prerhs")
                nc.vector.tensor_copy(out=rhs[:], in_=rhs32[:])
                psO = ps.tile([F, NT], mybir.dt.float32, tag="psO")
                nc.tensor.matmul(psO[:], prew_t[:], rhs[:], start=True, stop=True)
                dview = myout0[t * NT:(t + 1) * NT, :].rearrange("(m p) c -> p m c", p=128)
                dsts = [dview]
                if DEBUG:
                    dsts.append(dbg["pre"].ap()[t * NT:(t + 1) * NT, :]
                                .rearrange("(m p) c -> p m c", p=128))
                epilogue(psO, F, True, 4, dsts)
            nc.gpsimd.collective_compute(
                "AllGather", mybir.AluOpType.bypass, replica_groups=RG,
                ins=[myout0[0:NPAD[0], :]],
                outs=[tables["T0"][0:NCORES * NPAD[0], :]])

            # ---------------- gather-GEMM convs ----------------
            for (name, li, lo, tin, tout, cout, relu, isbg, isq) in specs:
                npo = NPAD[lo]
                ncol = npo // 128
                tbl = tables[tin]
                n_tiles = npo // NT
                idxt = cst.tile([128, NPAIR * n_tiles * 8], mybir.dt.int32, tag="idxt")
                nc.sync.dma_start(idxt[:], idx_in[name].ap())
                w2s = cst.tile([128, NPAIR * cout], mybir.dt.float32, tag="w2s")
                nc.sync.dma_start(w2s[:], w_in[name].ap())
                w2 = cst.tile([128, NPAIR * cout], mybir.dt.float32r, tag="w2")
                nc.vector.tensor_copy(out=w2[:], in_=w2s[:])
                biast = cst.tile([cout, 1], mybir.dt.float32, tag="bias")
                nc.sync.dma_start(biast[:], b_in[name].ap())

                for t in range(n_tiles):
                    psO = ps.tile([cout, NT], mybir.dt.float32, tag="psO")
                    for p in range(NPAIR):
                        base = (p * n_tiles + t) * 8
                        psTa = ps.tile([F, NT], mybir.dt.float32, tag="psTa")
                        psTb = ps.tile([F, NT], mybir.dt.float32, tag="psTb")
                        for m in range(4):
                            for half, psT in ((0, psTa), (1, psTb)):
                                g1 = sb.tile([128, F], mybir.dt.float32,
                                             tag="g1", bufs=16)
                                nc.gpsimd.indirect_dma_start(
                                    out=g1[:], out_offset=None, in_=tbl[:],
                                    in_offset=bass.IndirectOffsetOnAxis(
                                        ap=idxt[:, base + 2 * m + half:
                                                base + 2 * m + half + 1],
                                        axis=0))
                                nc.tensor.transpose(
                                    psT[:, m * 128:(m + 1) * 128],
                                    g1[:], id128[:])
                        rhs = sb.tile([128, NT], mybir.dt.float32r, tag="rhs")
                        nc.vector.tensor_copy(out=rhs[0:F, :], in_=psTa[:])
                        nc.vector.tensor_copy(out=rhs[F:128, :], in_=psTb[:])
                        nc.tensor.matmul(psO[:], w2[:, p * cout:(p + 1) * cout],
                                         rhs[:], start=(p == 0), stop=(p == NPAIR - 1))
                    # destinations
                    dests = []
                    if isbg:
                        dests.append(bg_out[name].ap()[t * NT:(t + 1) * NT, :]
                                     .rearrange("(m p) c -> p m c", p=128))
                    else:
                        dests.append(myout[lo][t * NT:(t + 1) * NT, :]
                                     .rearrange("(m p) c -> p m c", p=128))
                    if isq:
                        dests.append(q_out.ap()[t * NT:(t + 1) * NT, :]
                                     .rearrange("(m p) c -> p m c", p=128))
                    if DEBUG and name in dbg:
                        dests.append(dbg[name].ap()[t * NT:(t + 1) * NT, :]
                                     .rearrange("(m p) c -> p m c", p=128))
                    epilogue(psO, cout, relu, 4, dests)

                if tout is not None:
                    nc.gpsimd.collective_compute(
                        "AllGather", mybir.AluOpType.bypass, replica_groups=RG,
                        ins=[myout[lo][0:npo, :]],
                        outs=[tables[tout][0:NCORES * npo, :]])

    nc.compile()
    return nc


def _remap(idx, nl_in, np_in, zrow):
    """Original global row -> padded global row; absent -> zero row."""
    out = np.where(idx >= nl_in * NCORES, zrow,
                   (idx // nl_in) * np_in + (idx % nl_in))
    return out.astype(np.int32)


def _idx_input(idx, core, lvl_in, lvl_out):
    """Build [128, 14*n_tiles*8] int32 multi-index gather layout for one core.

    out[p, (pp*n_tiles + t)*8 + 2*m + half] = row for tap 2*pp+half,
    output voxel 512*t + 128*m + p, so that each gathered tile's
    [128, m*128:(m+1)*128] slice holds (tap-a chunk m | tap-b chunk m).
    """
    nl_o, np_o = NLOC[lvl_out], NPAD[lvl_out]
    nl_i, np_i = NLOC[lvl_in], NPAD[lvl_in]
    zrow = NCORES * np_i
    n_t = np_o // NT
    blk = idx[:, core * nl_o:(core + 1) * nl_o]
    full = np.full((28, np_o), zrow, np.int64)
    full[:27, :nl_o] = _remap(blk, nl_i, np_i, zrow)
    a = full.reshape(14, 2, n_t, 4, 128)          # [pp, half, t, m, p]
    a = a.transpose(4, 0, 2, 3, 1)                # [p, pp, t, m, half]
    return np.ascontiguousarray(a.reshape(128, NPAIR * n_t * 8).astype(np.int32))


def _w_input(W, cout):
    """W [27, 64(padded in-ch), cout] -> [128, 14*cout] two-tap-packed lhsT."""
    Wp = np.zeros((28, 64, cout), np.float32)
    Wp[:27] = W
    out = np.zeros((128, NPAIR * cout), np.float32)
    for p in range(NPAIR):
        out[0:64, p * cout:(p + 1) * cout] = Wp[2 * p]
        out[64:128, p * cout:(p + 1) * cout] = Wp[2 * p + 1]
    return out


def kernel(**inputs):
    from concourse import bass_utils

    if "nc" not in _CACHE:
        _CACHE["nc"] = _build()
    nc = _CACHE["nc"]

    f32 = lambda a: np.asarray(a, np.float32)
    x = f32(inputs["x"])
    idxs = {k: np.asarray(inputs[k], np.int64) for k in
            ["idx0", "idx_d0", "idx1", "idx_d1", "idx2", "idx_d2", "idx3"]}
    idx_for = {"down0": ("idx_d0", 0, 1), "c1_0": ("idx1", 1, 1), "c2_0": ("idx1", 1, 1),
               "p1_0": ("idx1", 1, 1), "p2_0": ("idx1", 1, 1),
               "down1": ("idx_d1", 1, 2), "c1_1": ("idx2", 2, 2), "c2_1": ("idx2", 2, 2),
               "p1_1": ("idx2", 2, 2), "p2_1": ("idx2", 2, 2),
               "down2": ("idx_d2", 2, 3), "c1_2": ("idx3", 3, 3), "c2_2": ("idx3", 3, 3),
               "p1_2": ("idx3", 3, 3), "p2_2": ("idx3", 3, 3)}

    dW, dB = {}, {}
    for i, lvl in enumerate(["0", "1", "2"]):
        dW[f"down{lvl}"] = f32(inputs["down_W"][i]); dB[f"down{lvl}"] = f32(inputs["down_b"][i])
        dW[f"c1_{lvl}"] = f32(inputs["conv1_W"][i]); dB[f"c1_{lvl}"] = f32(inputs["conv1_b"][i])
        dW[f"c2_{lvl}"] = f32(inputs["conv2_W"][i]); dB[f"c2_{lvl}"] = f32(inputs["conv2_b"][i])
        p1w = np.zeros((27, F, F), np.float32); p1w[:, :, :S] = f32(inputs["pred1_W"][i])
        p1b = np.zeros((F,), np.float32); p1b[:S] = f32(inputs["pred1_b"][i])
        dW[f"p1_{lvl}"] = p1w; dB[f"p1_{lvl}"] = p1b
        p2w = np.zeros((27, F, S), np.float32); p2w[:, :S, :] = f32(inputs["pred2_W"][i])
        dW[f"p2_{lvl}"] = p2w; dB[f"p2_{lvl}"] = f32(inputs["pred2_b"][i])

    # host-side pre-gather for the pre conv: X27T [128, NPAD0] per core
    xpad = np.zeros((NLVL[0] + 1, 4), np.float32)
    xpad[:NLVL[0]] = x
    idx0 = idxs["idx0"]  # [27, 256000], absent == 256000
    preW = f32(inputs["pre_W"])  # [27, 4, 64]
    prew_in = np.zeros((128, F), np.float32)
    prew_in[:108] = preW.reshape(108, F)
    preb_in = f32(inputs["pre_b"]).reshape(F, 1)

    in_maps = []
    for c in range(NCORES):
        m = {"prew": prew_in, "preb": preb_in}
        g = xpad[idx0[:, c * NLOC[0]:(c + 1) * NLOC[0]]]      # [27, 32000, 4]
        x27 = np.zeros((128, NPAD[0]), np.float32)
        x27[:108, :NLOC[0]] = g.transpose(0, 2, 1).reshape(108, NLOC[0])
        m["x27"] = x27
        for name, (iname, li, lo) in idx_for.items():
            m[f"idx_{name}"] = _idx_input(idxs[iname], c, li, lo)
            cout = S if name.startswith("p2") else F
            m[f"w_{name}"] = _w_input(dW[name], cout)
            m[f"b_{name}"] = dB[name].reshape(cout, 1)
        in_maps.append(m)

    res = bass_utils.run_bass_kernel_spmd(nc, in_maps, core_ids=list(range(NCORES)))
    r = res.results

    Q = np.concatenate([r[c]["q_out"][:NLOC[3]] for c in range(NCORES)])
    bg0 = np.concatenate([r[c]["bg0_out"][:NLOC[1]] for c in range(NCORES)])
    bg1 = np.concatenate([r[c]["bg1_out"][:NLOC[2]] for c in range(NCORES)])
    bg2 = np.concatenate([r[c]["bg2_out"][:NLOC[3]] for c in range(NCORES)])
    return (Q, bg0, bg1, bg2)
